# revision 22
# baseline (speedup 1.0000x reference)
"""Bass/Tile TRN2 kernel for nn_BayesHead (projected single-head attention,
near-causal mask tril(diag=1), double 1/sqrt(64) scaling).

Strategy (8 NeuronCores, pure data-parallel SPMD — no collectives):
  - core j handles batch b = j//2 with key-parity p = j%2.
  - Each core projects ALL 4096 queries of its batch, and its HALF of the
    keys/values (interleaved 128-row blocks: global block g = 2*sigma + p).
  - Flash-style partial softmax without max-subtraction (scores are in
    [-1,1] after the 1/64 scaling, so exp is safe): each core produces
    O_p[h, t] = sum_{s in its keys, s <= t+1} exp(S) * V[s, h] plus a
    denominator row (ones-column trick).  The host sums the two partials
    per batch and normalizes.

v2 scheduling (vs the 109us baseline):
  - DRAM inputs pre-laid-out as [128, ct, t] so one dma_start per 512-col
    chunk lands contiguously; chunks issued in exact consumption order so
    all 16 DMA engines run from t~0 and compute starts at ~4us.
  - Mask tensors built on the (otherwise idle) GPSIMD engine so the DVE
    stream never blocks early PSUM-evacuation copies.
  - Projections interleaved with attention tiles so the PE stays
    continuously busy (HAM ramps to 2.4 GHz) while the ACT engine chews
    the exp stream in parallel.
  - The last s-tile of each query tile is >99% masked (only its first key
    is visible, to the last query): scores/exp/mask/PV are trimmed to the
    final 128 columns there.
"""

import numpy as np
from contextlib import ExitStack

import concourse.bass as bass
import concourse.mybir as mybir
import concourse.tile as tile
from concourse import bacc
from concourse.bass import ts
from concourse.bass_utils import run_bass_kernel_spmd

B, T, C, H = 4, 4096, 1024, 64
NCORES = 8
TQ = 512                       # query-tile width
NQT = T // TQ                  # 8 query tiles
NSB = (T // 2) // 128          # 16 local key tiles (128 rows each)
NCT = C // 128                 # 8 contraction tiles
TH = T // 2
# s-tile capacity per query tile (identical for both parities; covers causal
# reach ceil((4i+5)/2), capped at the 16 local tiles)
CAPS = [min(NSB, 2 * i + 3) for i in range(NQT)]
MASK_FROM = [2 * i for i in range(NQT)]  # sigma >= 2i may cross the diagonal
# The mask for tile (i, s) depends only on e = 2s - 4i in {0, 2, 4}:
# thr = 128*(2s+p) + r - 512i - 1 = 128*e + 128*p + r - 1.  Three masks total.
M_IDX = {(i, s): (2 * s - 4 * i) // 2
         for i in range(NQT) for s in range(MASK_FROM[i], CAPS[i])}
N_MASKED = 3
W0 = 480                       # live-column window start for singleton s-tiles
# (p0 singleton has exactly 1 live col (511); p1 singleton is fully dead)
FP = mybir.dt.float16
F32 = mybir.dt.float32
SCALE = 1.0 / H                # (H**-0.5) applied twice


def build_bass():
    nc = bacc.Bacc("TRN2", target_bir_lowering=False, num_devices=NCORES)
    # DRAM layouts are pre-transposed on host and chunk-major:
    # x[p, chunk, ct, col] = x.T[128*ct+p, 512*chunk+col], so each 512-col
    # chunk DMA moves 8KB contiguous per partition (128 fat descriptors)
    qT = nc.declare_dram_parameter("qT", [128, NQT, NCT, 512], FP, isOutput=False)
    kT = nc.declare_dram_parameter("kT", [128, NQT // 2, NCT, 512], FP, isOutput=False)
    vT = nc.declare_dram_parameter("vT", [128, NQT // 2, NCT, 512], FP, isOutput=False)
    wq = nc.declare_dram_parameter("wq", [128, NCT, H], FP, isOutput=False)
    wk = nc.declare_dram_parameter("wk", [128, NCT, H], FP, isOutput=False)
    wv = nc.declare_dram_parameter("wv", [128, NCT, H], FP, isOutput=False)
    iota = nc.declare_dram_parameter("iota", [128, TQ], FP, isOutput=False)
    thr = nc.declare_dram_parameter("thr", [128, N_MASKED], F32, isOutput=False)
    ident = nc.declare_dram_parameter("ident", [64, 64], FP, isOutput=False)
    out = nc.declare_dram_parameter("out", [H + 1, T], F32, isOutput=True)

    with ExitStack() as ctx:
        tc = ctx.enter_context(tile.TileContext(nc))
        singles = ctx.enter_context(tc.tile_pool(name="singles", bufs=1))
        pt_pool = ctx.enter_context(tc.tile_pool(name="pt", bufs=6))
        outsb_pool = ctx.enter_context(tc.tile_pool(name="outsb", bufs=6))
        stage_pool = ctx.enter_context(tc.tile_pool(name="stage", bufs=2))
        psum_s = ctx.enter_context(tc.tile_pool(name="psum_s", bufs=3, space="PSUM"))
        psum_o = ctx.enter_context(tc.tile_pool(name="psum_o", bufs=2, space="PSUM"))

        # SBUF-resident tiles
        iota_sb = singles.tile([128, TQ], FP)
        thr_sb = singles.tile([128, N_MASKED], F32)
        wq_sb = singles.tile([128, NCT, H], FP)
        wk_sb = singles.tile([128, NCT, H], FP)
        wv_sb = singles.tile([128, NCT, H], FP)
        id_sb = singles.tile([64, 64], FP)
        q_sb = singles.tile([128, NQT, NCT, 512], FP)
        k_sb = singles.tile([128, NQT // 2, NCT, 512], FP)
        v_sb = singles.tile([128, NQT // 2, NCT, 512], FP)

        qp_sb = singles.tile([128, T], FP)        # Q^T [h, t], dup on parts 64-127
        kp_sb = singles.tile([128, TH], FP)       # K^T [h, s], dup on parts 64-127
        va_sb = singles.tile([128, NSB, H + 1], FP)  # V rows [s, h] + ones col
        masks_sb = singles.tile([128, N_MASKED, TQ], FP)

        # ---- DMA issue stream (sync engine), deadline order ----
        # Arrival pacing is ~2.85us/MB; the exp (ACT) stream is paced by q_i
        # arrivals early on, so q chunks go as early as k/v deadlines allow.
        def dq(c):
            nc.sync.dma_start(out=q_sb[:, c, :, :], in_=qT[:, c, :, :])

        def dk(c, c0=0, c1=512):
            nc.sync.dma_start(out=k_sb[:, c, :, c0:c1], in_=kT[:, c, :, c0:c1])

        def dv(c, c0=0, c1=512):
            nc.sync.dma_start(out=v_sb[:, c, :, c0:c1], in_=vT[:, c, :, c0:c1])

        nc.sync.dma_start(out=wq_sb, in_=wq[:, :, :])
        dq(0)
        dq(1)
        nc.sync.dma_start(out=wk_sb, in_=wk[:, :, :])
        dk(0)
        dk(1, 0, 256)                      # s4-5
        nc.sync.dma_start(out=wv_sb, in_=wv[:, :, :])
        dv(0)
        nc.sync.dma_start(out=thr_sb, in_=thr[:, :])
        nc.sync.dma_start(out=iota_sb, in_=iota[:, :])
        nc.sync.dma_start(out=id_sb, in_=ident[:, :])
        dq(2)
        dq(3)
        dk(1, 256, 512)                    # s6-7
        dv(1)
        dk(2, 0, 256)                      # s8-9
        dq(4)
        dq(5)
        dk(2, 256, 512)                    # s10-11
        dv(2)
        dk(3, 0, 256)                      # s12-13
        dq(6)
        dq(7)
        dk(3, 256, 512)                    # s14-15
        dv(3)

        # ones column for the softmax denominator
        nc.vector.memset(va_sb[:, :, H:H + 1], 1.0)



        def build_masks():
            # Only 3 distinct masks exist (e = 2s-4i in {0,2,4}); build once.
            for m in range(N_MASKED):
                nc.vector.tensor_scalar(
                    masks_sb[:, m, :], iota_sb[:, :], thr_sb[:, m:m + 1],
                    None, mybir.AluOpType.is_ge)

        def q_proj(tq):
            pq = psum_s.tile([128, 512], F32, tag="ps")
            for ct in range(NCT):
                nc.tensor.matmul(pq[0:64, :], wq_sb[:, ct, :],
                                 q_sb[:, tq, ct, :], tile_position=(0, 0),
                                 start=(ct == 0), stop=(ct == NCT - 1))
                nc.tensor.matmul(pq[64:128, :], wq_sb[:, ct, :],
                                 q_sb[:, tq, ct, :], tile_position=(0, 64),
                                 start=(ct == 0), stop=(ct == NCT - 1),
                                 skip_group_check=True)
            nc.vector.tensor_copy(qp_sb[:, ts(tq, 512)], pq)

        def q_proj_pair(ta, tb):
            # Two q-tiles share one slot as independent col tiles; the
            # partition-dup (needed by the row-tiled scores) comes from DVE
            # copies instead of a redundant second matmul.
            pq = psum_s.tile([128, 512], F32, tag="ps")
            for ct in range(NCT):
                nc.tensor.matmul(pq[0:64, :], wq_sb[:, ct, :],
                                 q_sb[:, ta, ct, :], tile_position=(0, 0),
                                 start=(ct == 0), stop=(ct == NCT - 1))
                nc.tensor.matmul(pq[64:128, :], wq_sb[:, ct, :],
                                 q_sb[:, tb, ct, :], tile_position=(0, 64),
                                 start=(ct == 0), stop=(ct == NCT - 1),
                                 skip_group_check=True)
            nc.vector.tensor_copy(qp_sb[0:64, ts(ta, 512)], pq[0:64, :])
            nc.vector.tensor_copy(qp_sb[64:128, ts(ta, 512)], pq[0:64, :])
            nc.vector.tensor_copy(qp_sb[0:64, ts(tb, 512)], pq[64:128, :])
            nc.vector.tensor_copy(qp_sb[64:128, ts(tb, 512)], pq[64:128, :])

        def k_proj(c4, c0=0, c1=512):
            pk = psum_s.tile([128, 512], F32, tag="ps")
            for ct in range(NCT):
                nc.tensor.matmul(pk[0:64, c0:c1], wk_sb[:, ct, :],
                                 k_sb[:, c4, ct, c0:c1], tile_position=(0, 0),
                                 start=(ct == 0), stop=(ct == NCT - 1))
                nc.tensor.matmul(pk[64:128, c0:c1], wk_sb[:, ct, :],
                                 k_sb[:, c4, ct, c0:c1], tile_position=(0, 64),
                                 start=(ct == 0), stop=(ct == NCT - 1),
                                 skip_group_check=True)
            nc.vector.tensor_copy(kp_sb[:, 512 * c4 + c0:512 * c4 + c1],
                                  pk[:, c0:c1])

        def v_proj(c4, j0=0, j1=4):
            cols = slice(128 * j0, 128 * j1)
            pv = psum_s.tile([64, 512], F32, tag="ps")
            for ct in range(NCT):
                nc.tensor.matmul(pv[:, cols], wv_sb[:, ct, :],
                                 v_sb[:, c4, ct, cols],
                                 start=(ct == 0), stop=(ct == NCT - 1))
            vt_stage = stage_pool.tile([64, 512], FP)
            nc.vector.tensor_copy(vt_stage[:, cols], pv[:, cols])
            for j in range(j0, j1):
                sig = c4 * 4 + j
                ptr = psum_o.tile([128, H], FP, tag="oacc")
                nc.tensor.transpose(ptr, vt_stage[:, ts(j, 128)], id_sb)
                nc.vector.tensor_copy(va_sb[:, sig, 0:H], ptr)

        carry = []                 # deferred tail-PV flush of the previous tile

        def attention(i, mid=None, mid_leads=None):
            cap = CAPS[i]
            po = psum_o.tile([H + 1, 512], F32, tag="oacc")
            # group list: leading full pairs, then (i<7) the nearly-dead
            # singleton (trimmed to cols [W0,512)), then the diagonal pair
            # (2i, 2i+1) last so the accumulation stop lands on a full-width
            # matmul.
            groups = []
            lead = cap if i == 7 else 2 * i
            for g0 in range(0, lead, 2):
                groups.append(("pair", g0))
            if i == 0:
                # the first PV writing po must be full width (PSUM zero-region
                # start semantics), so the trimmed singleton goes last
                groups = [("pair", 0), ("single", 2)]
            elif i < 7:
                groups.append(("single", cap - 1))
                groups.append(("pair", 2 * i))
            def emit_s(kind, g0):
                # scores matmuls + exp + mask for one group; returns pt handle
                if kind == "pair":
                    ps = psum_s.tile([128, 1024], F32, tag="ps")
                    for g in (0, 1):
                        sig = g0 + g
                        nc.tensor.matmul(ps[:, ts(g, 512)],
                                         kp_sb[ts(g, 64), ts(sig, 128)],
                                         qp_sb[ts(g, 64), ts(i, 512)],
                                         tile_position=(64 * g, 0),
                                         start=True, stop=True)
                    pt = pt_pool.tile([128, 1024], FP)
                    nc.scalar.activation(pt, ps,
                                         mybir.ActivationFunctionType.Exp,
                                         scale=SCALE)
                    for g in (0, 1):
                        sig = g0 + g
                        if sig >= MASK_FROM[i]:
                            m = M_IDX[(i, sig)]
                            nc.vector.tensor_mul(pt[:, ts(g, 512)],
                                                 pt[:, ts(g, 512)],
                                                 masks_sb[:, m, :])
                else:  # singleton: full-width scores (PSUM zero-region rule),
                    # but exp/mask/PV trimmed to the live cols [W0:512)
                    sig = g0
                    m = M_IDX[(i, sig)]
                    ps = psum_s.tile([128, 512], F32, tag="ps")
                    nc.tensor.matmul(ps,
                                     kp_sb[0:64, ts(sig, 128)],
                                     qp_sb[0:64, ts(i, 512)],
                                     tile_position=(0, 0), start=True, stop=True)
                    pt = pt_pool.tile([128, 512], FP)
                    nc.scalar.activation(pt[:, W0:512], ps[:, W0:512],
                                         mybir.ActivationFunctionType.Exp,
                                         scale=SCALE)
                    nc.vector.tensor_mul(pt[:, W0:512], pt[:, W0:512],
                                         masks_sb[:, m, W0:512])
                return pt

            state = {"first": True}

            def emit_pv(kind, g0, pt, last_grp):
                if kind == "pair":
                    for g in (0, 1):
                        sig = g0 + g
                        nc.tensor.matmul(po, va_sb[:, sig, :], pt[:, ts(g, 512)],
                                         start=state["first"],
                                         stop=(last_grp and g == 1))
                        state["first"] = False
                else:
                    nc.tensor.matmul(po[:, W0:512], va_sb[:, g0, :],
                                     pt[:, W0:512], start=state["first"],
                                     stop=last_grp)
                    state["first"] = False

            # software pipeline: PV stream lags the scores stream by 3 groups
            # so the PE never stalls on the exp+mask latency.  The tail PVs
            # (which wait on the tile's last exps) are NOT flushed here: they
            # carry over and are emitted inside the NEXT attention, after its
            # first score groups, so the PE never idles on the exp boundary.
            LAG = 3
            pend = []
            for gi, (kind, g0) in enumerate(groups):
                if kind == "single" and mid_leads is not None:
                    # k-projection feeding this singleton (its DMA chunk
                    # arrives later than the lead pairs' data)
                    mid_leads()
                pt = emit_s(kind, g0)
                if gi == min(1, len(groups) - 1) and carry:
                    carry.pop()()      # flush previous tile's tail PVs + out
                pend.append((kind, g0, pt))
                if gi >= LAG:
                    k_, g_, pt_ = pend.pop(0)
                    emit_pv(k_, g_, pt_, last_grp=False)
            if mid is not None:
                # next tile's projections go here so the PE chews them while
                # the ACT engine finishes this tile's trailing exps
                mid()

            def flush():
                for j, (k_, g_, pt_) in enumerate(pend):
                    emit_pv(k_, g_, pt_, last_grp=(j == len(pend) - 1))
                osb = outsb_pool.tile([H + 1, 512], F32)
                nc.vector.tensor_copy(osb, po)
                nc.sync.dma_start(out=out[:, ts(i, 512)], in_=osb)
            carry.append(flush)

        # ---- compute schedule: deadline-aligned with the DMA stream ----
        # attention(i) consumes qp_i, kp s-tiles <= 2i+2, va s-tiles <= 2i+2.
        # Each attention's mid-hook carries upcoming projections so they
        # overlap that tile's trailing exp stream on the ACT engine.
        build_masks()
        q_proj_pair(0, 1)
        k_proj(0)                  # s0-3
        attention(0, mid=lambda: (v_proj(0), k_proj(1, 0, 256)))
        attention(1, mid=lambda: (q_proj(2), v_proj(1, 0, 2)))
        attention(2, mid_leads=lambda: k_proj(1, 256, 512),
                  mid=lambda: (q_proj(3), v_proj(1, 2, 4)))
        attention(3, mid_leads=lambda: k_proj(2, 0, 256),
                  mid=lambda: (q_proj(4), v_proj(2, 0, 2)))
        attention(4, mid_leads=lambda: k_proj(2, 256, 512),
                  mid=lambda: (q_proj(5), v_proj(2, 2, 4)))
        attention(5, mid_leads=lambda: k_proj(3, 0, 256),
                  mid=lambda: (q_proj(6), v_proj(3, 0, 2)))
        attention(6, mid_leads=lambda: k_proj(3, 256, 512),
                  mid=lambda: (q_proj(7), v_proj(3, 2, 4)))
        attention(7)
        carry.pop()()              # final tile's tail PVs + output

    nc.compile()
    return nc


_NC = None


def _get_nc():
    global _NC
    if _NC is None:
        _NC = build_bass()
    return _NC


def _prep_core_inputs(q, k, v, Wq, Wk, Wv):
    f2 = np.float16

    def wprep(W):
        # SBUF layout [p, ct, h] = W.T[ct*128+p, h]
        return np.ascontiguousarray(W.T.reshape(NCT, 128, H).transpose(1, 0, 2)).astype(f2)

    def xprep(x):
        # [p, chunk, ct, col] = x.T[128*ct+p, 512*chunk+col]
        xt = x.T.astype(f2)                       # [C, T']
        nch = xt.shape[1] // 512
        return np.ascontiguousarray(
            xt.reshape(NCT, 128, nch, 512).transpose(1, 2, 0, 3))

    wq_h, wk_h, wv_h = wprep(Wq), wprep(Wk), wprep(Wv)
    iota_h = np.ascontiguousarray(
        np.broadcast_to(np.arange(TQ, dtype=np.float32), (128, TQ))).astype(f2)
    ident_h = np.eye(64, dtype=f2)

    r = np.arange(128)
    in_maps = []
    for j in range(NCORES):
        b, p = j // 2, j % 2
        rows = (np.arange(TH) // 128) * 256 + p * 128 + (np.arange(TH) % 128)
        qT_h = xprep(q[b])
        kT_h = xprep(k[b][rows])
        vT_h = xprep(v[b][rows])
        thr_h = np.empty((128, N_MASKED), np.float32)
        for m in range(N_MASKED):
            t = 256 * m + 128 * p + r - 1
            thr_h[:, m] = np.clip(t, -1024, 1024).astype(np.float32)
        in_maps.append({
            "qT": qT_h, "kT": kT_h, "vT": vT_h,
            "wq": wq_h, "wk": wk_h, "wv": wv_h,
            "iota": iota_h, "thr": thr_h, "ident": ident_h,
        })
    return in_maps


def _run(inputs, trace=False, trace_kwargs=None):
    nc = _get_nc()
    in_maps = _prep_core_inputs(
        inputs["q"], inputs["k"], inputs["v"],
        inputs["Wq"], inputs["Wk"], inputs["Wv"])
    res = run_bass_kernel_spmd(nc, in_maps, list(range(NCORES)), trace=trace,
                               **(trace_kwargs or {}))
    outs = [res.results[j]["out"] for j in range(NCORES)]
    y = np.empty((B, T, H), np.float32)
    for b in range(B):
        s = outs[2 * b] + outs[2 * b + 1]      # [H+1, T]
        y[b] = (s[:H] / s[H:H + 1]).T
    return y, res


def kernel(q, k, v, Wq, Wk, Wv):
    y, _ = _run({"q": np.asarray(q), "k": np.asarray(k), "v": np.asarray(v),
                 "Wq": np.asarray(Wq), "Wk": np.asarray(Wk), "Wv": np.asarray(Wv)})
    return y

